# revision 2
# baseline (speedup 1.0000x reference)
"""Contrastive projection head loss on 8 Trainium2 NeuronCores (v8, 219us; v2 baseline was 250-279us).

Reference computation (B=8192, E=1024, P=512):
    z_codon = relu(x[:, :E]) @ w + b          # [B, P]
    z_amino = relu(x[:, E:]) @ w + b          # [B, P]
    z  = concat([z_codon, z_amino], axis=1)   # [B, 2P]
    zn = z / max(||z||, 1e-8)
    s  = (zn @ zn.T);  s[i,i] = -9e15;  s /= 0.1
    nll_i = -s[i, (i - B/2) % B] + logsumexp(s[i, :])
    out = mean(nll)

v8 changes vs v7:
 - Remote pass order k0(dd1-3), k0(dd4), k1(dd1-3), k1(dd4): all the
   AG0-gated work runs before anything needs AG1, removing the ~8us
   PE stall observed while AG1 was still in flight.

v7 changes vs v6:
 - w staged on the sync ring BETWEEN x row-groups 5 and 6: rows 0-3
   stream uncontended so the jh0 chain (and AG0) launch ~12us earlier;
   w still lands just before the jh0 projections need it.
 - Column-sum exchange split back into two AllGathers, each launched
   right after its k-pass so both hide under the dd=4 passes.

v6 changes vs v5:
 - Dummy 128B AllGather right after the load triggers: the ~14us
   core-launch skew drains during the x stream instead of inflating
   the first real collective.
 - Activation-table warmups removed (tables evict on function switch,
   so they bought nothing and cost ~4us of early scalar time).

v5 changes vs v4 (261us) / v3 (225us):
 - ALL x DMA triggers on the sync queue (one ring sustains the ~260GB/s
   per-core HBM rate; putting half on scalar let ring backpressure
   block scalar compute behind the trigger slices until ~25us).
 - xrow pool bufs=8: no buffer-reuse waits.
 - Back to two per-half 512KB AllGathers (the single 1MB AG cost 43us
   of CC time and stalled the PE 68us; two small ones pipeline).
 - PSUM transpose staging [128, 16, 128] (2 banks): one eviction copy
   per row-group.
 - Engine split tuned to measured rates: relu V/S alternating, big
   copies V/S alternating, sq and normalize Vector/GpSimd alternating.

Returns per-core partial sums [1, 8]; host sums and divides by B.
"""
import numpy as np

from concourse import bass, mybir, tile, bacc
from concourse.bass_utils import run_bass_kernel_spmd
from concourse.masks import make_identity

N_CORES = 8
B = 8192
E = 1024          # embedding size (per half)
P = 512           # projection size
D = 2 * P         # z feature dim = 1024
R = B // N_CORES  # rows per core = 1024
KT = D // 128     # feature sub-tiles = 8
MT = R // 128     # row sub-tiles per core = 8
INV_T = 10.0      # 1 / temperature
SC_Z = 16.0       # zn pre-scale before fp8 cast
SC_W = 32.0       # w pre-scale before fp8 cast
EXP_SC = INV_T / (SC_Z * SC_Z)   # activation scale recovering s/T

F32 = mybir.dt.float32
F32R = mybir.dt.float32r
BF16 = mybir.dt.bfloat16
FP8 = mybir.dt.float8e4
FP8E5 = mybir.dt.float8e5
AF = mybir.ActivationFunctionType
ALU = mybir.AluOpType
DR = mybir.MatmulPerfMode.DoubleRow

NSLOT = 10        # rowsum slots: 2 local (d=0) + 8 remote (d=1..4, k=0..1)

_cached = {}


def _build(no_collective=False):
    nc = bacc.Bacc("TRN2", target_bir_lowering=False, debug=False,
                   enable_asserts=False, num_devices=N_CORES)
    x_in = nc.dram_tensor("xs", [R, 2 * E], F32, kind="ExternalInput").ap()
    w_in = nc.dram_tensor("w", [E, P], F32, kind="ExternalInput").ap()
    b_in = nc.dram_tensor("b", [P], F32, kind="ExternalInput").ap()
    out = nc.dram_tensor("out", [1, MT], F32, kind="ExternalOutput").ap()

    with tile.TileContext(nc) as tc:
        with tc.tile_pool(name="const", bufs=1) as const, \
             tc.tile_pool(name="big", bufs=1) as big, \
             tc.tile_pool(name="small", bufs=1) as small, \
             tc.tile_pool(name="dram", bufs=1, space="DRAM") as dram:

            ident = const.tile([128, 128], F32)
            make_identity(nc, ident[:])
            idb = const.tile([128, 128], BF16)
            make_identity(nc, idb[:])
            ones_f = const.tile([128, 1], F32)
            nc.vector.memset(ones_f[:], 1.0)
            ones_r = const.tile([128, 1], F32R)
            nc.vector.tensor_copy(ones_r[:], ones_f[:])
            ones_f1 = const.tile([1, 128], F32)
            nc.vector.memset(ones_f1[:], 1.0)
            ones_r1 = const.tile([1, 128], F32R)
            nc.vector.tensor_copy(ones_r1[:], ones_f1[:])
            ones_e52 = const.tile([128, 2, 128], FP8E5)
            nc.vector.memset(ones_e52[:], 1.0)
            b2 = const.tile([128, P // 128], F32)
            nc.sync.dma_start(b2[:], b_in.rearrange("(mt p) -> p mt", p=128))

            # w as [128, KT(=E/128), P] fp8, scaled x32
            w8 = const.tile([128, E // 128, P], FP8)

            # znT8: zn x16, fp8, jh-major feature-major [128, 2, KT, 512]
            znT8 = big.tile([128, 2, KT, 512], FP8, tag="zn8")
            ag_in = [dram.tile([128, KT * 512], FP8, name=f"ag_in{k}")
                     for k in range(2)]
            ag_out = [dram.tile([N_CORES * 128, KT * 512], FP8,
                                name=f"ag_out{k}",
                                addr_space="Local" if no_collective else "Shared")
                      for k in range(2)]
            dummy_in = dram.tile([1, 128], FP8, name="dummy_in")
            dummy_out = dram.tile([N_CORES, 128], FP8, name="dummy_out",
                                  addr_space="Local" if no_collective
                                  else "Shared")
            cs_in = [dram.tile([3, 512], F32, name=f"cs_in{k}")
                     for k in range(2)]
            cs_out = [dram.tile([N_CORES * 3, 512], F32, name=f"cs_out{k}",
                                addr_space="Local" if no_collective
                                else "Shared")
                      for k in range(2)]

            # ---- phase 1 ----
            with tc.tile_pool(name="xrow", bufs=8) as xrowp, \
                 tc.tile_pool(name="wst", bufs=1) as wstp, \
                 tc.tile_pool(name="x16", bufs=3) as x16p, \
                 tc.tile_pool(name="xT", bufs=2) as xTp, \
                 tc.tile_pool(name="zT", bufs=2) as zTp, \
                 tc.tile_pool(name="sq", bufs=2) as sqp, \
                 tc.tile_pool(name="rns", bufs=4) as rnsp, \
                 tc.tile_pool(name="ps1", bufs=1, space="PSUM") as ps1:

                # all x loads queued up front; 8 buffers so no trigger
                # ever waits on compute.
                xrows = []
                wstage = wstp.tile([128, E // 128, P], F32, tag="wstage")
                for rg in range(8):
                    xrow = xrowp.tile([128, 2 * E], F32, tag="xrow")
                    nc.sync.dma_start(xrow[:],
                                      x_in[rg * 128:(rg + 1) * 128, :])
                    xrows.append(xrow)
                    if rg == 5:
                        # w lands after rows 0-5, just before the jh0
                        # projections need it
                        nc.sync.dma_start(
                            wstage[:],
                            w_in.rearrange("(kt p) q -> p kt q", p=128))
                pid = nc.sync.partition_id()
                # dummy collective: absorbs core-launch skew while the x
                # stream runs, so the first real AllGather starts promptly
                if not no_collective:
                    nc.gpsimd.collective_compute(
                        "AllGather", ALU.bypass,
                        replica_groups=[list(range(N_CORES))],
                        ins=[dummy_in[:]], outs=[dummy_out[:]])

                for jh in range(2):
                    xT8 = xTp.tile([128, 2 * KT, 512], FP8, tag="xT",
                                   name=f"xT{jh}")
                    for r in range(4):
                        rg = jh * 4 + r
                        x16 = x16p.tile([128, 2 * E], BF16, tag="x16")
                        if rg % 2 == 0:
                            nc.vector.tensor_scalar_max(x16[:], xrows[rg][:],
                                                        0.0)
                        else:
                            nc.scalar.activation(x16[:], xrows[rg][:], AF.Relu)
                        # all 16 feature tiles transpose into one 2-bank
                        # psum tile; a single eviction copy per row-group
                        pt = ps1.tile([128, 16, 128], BF16, tag="pt",
                                      bufs=2)
                        for ct in range(16):
                            nc.tensor.transpose(
                                pt[:, ct, :],
                                x16[:, ct * 128:(ct + 1) * 128],
                                idb[:])
                        dst = xT8[:, :, r * 128:(r + 1) * 128]
                        if rg % 2 == 0:
                            nc.vector.tensor_copy(dst, pt[:])
                        else:
                            nc.scalar.activation(dst, pt[:], AF.Identity)
                    if jh == 0:
                        nc.vector.tensor_scalar_mul(w8[:], wstage[:], SC_W)
                    # project this half with DoubleRow fp8; interleave the
                    # norm accumulation so sqrt can fire right after the
                    # last projection tile.
                    zT = zTp.tile([128, KT, 512], F32, tag="zT",
                                  name=f"zT{jh}")
                    pn = ps1.tile([1, 512], F32, tag="pn", bufs=1,
                                  name=f"pn{jh}")
                    for h in range(2):
                        for m4 in range(P // 128):
                            pz = ps1.tile([128, 512], F32, tag="pz", bufs=2)
                            for t in range(4):
                                nc.tensor.matmul(
                                    pz[:],
                                    w8[:, 2 * t:2 * t + 2,
                                       m4 * 128:(m4 + 1) * 128],
                                    xT8[:, h * KT + 2 * t:h * KT + 2 * t + 2,
                                        :],
                                    start=(t == 0), stop=(t == 3),
                                    perf_mode=DR)
                            kt = h * 4 + m4
                            nc.scalar.activation(
                                zT[:, kt, :], pz[:], AF.Identity,
                                bias=b2[:, m4:m4 + 1], scale=1.0 / SC_W)
                            sq = sqp.tile([128, 512], F32R, tag="sq")
                            eng = nc.vector if kt % 2 == 0 else nc.gpsimd
                            eng.tensor_tensor(sq[:], zT[:, kt, :],
                                              zT[:, kt, :], ALU.mult)
                            nc.tensor.matmul(pn[:], ones_r[:], sq[:],
                                             start=(kt == 0), stop=(kt == 7))
                    nrm = rnsp.tile([1, 512], F32, tag="nrm", name=f"nrm{jh}")
                    nc.scalar.activation(nrm[:], pn[:], AF.Sqrt,
                                         scale=1.0 / (SC_Z * SC_Z))
                    rn = rnsp.tile([1, 512], F32, tag="rn", name=f"rn{jh}")
                    nc.vector.reciprocal_approx_fast(rn[:], nrm[:])
                    rnr = rnsp.tile([1, 512], F32R, tag="rnr", name=f"rnr{jh}")
                    nc.vector.tensor_copy(rnr[:], rn[:])
                    # broadcast rn across partitions on the PE (K=1 matmul)
                    rnp = ps1.tile([128, 512], F32, tag="rnp", bufs=1,
                                   name=f"rnp{jh}")
                    nc.tensor.matmul(rnp[:], ones_r1[:], rnr[:],
                                     start=True, stop=True)
                    # GpSimd can't read PSUM; land the broadcast in SBUF
                    rnb = rnsp.tile([128, 512], F32, tag="rnb",
                                    name=f"rnb{jh}")
                    nc.vector.tensor_copy(rnb[:], rnp[:])
                    for kt in range(KT):
                        eng = nc.vector if kt % 2 == 0 else nc.gpsimd
                        eng.tensor_tensor(znT8[:, jh, kt, :], zT[:, kt, :],
                                          rnb[:], ALU.mult)
                    # ship this half and gather while the rest computes
                    nc.scalar.dma_start(
                        ag_in[jh].rearrange("p (kt j) -> p kt j", kt=KT),
                        znT8[:, jh])
                    if no_collective:
                        for c in range(N_CORES):
                            nc.sync.dma_start(
                                ag_out[jh][c * 128:(c + 1) * 128, :],
                                ag_in[jh][:])
                    else:
                        nc.gpsimd.collective_compute(
                            "AllGather", ALU.bypass,
                            replica_groups=[list(range(N_CORES))],
                            ins=[ag_in[jh][:]], outs=[ag_out[jh][:]])

            # ---- phase 2: symmetric blockwise cos-sim ----
            rowsum = const.tile([128, MT, NSLOT], F32)
            pos_acc = const.tile([128, MT], F32)
            corr_acc = const.tile([128, MT], F32)

            def stat(m, t):
                return znT8[:, m // 4, 2 * t:2 * t + 2,
                            (m % 4) * 128:(m % 4 + 1) * 128]

            def dr_gemm(pg, m, rhs_ap, t):
                nc.tensor.matmul(pg[:], stat(m, t), rhs_ap,
                                 start=(t == 0), stop=(t == 3), perf_mode=DR)

            def diag_to(dst_ap, pg, m, scale_exp, junkp, dtmpp):
                off = (m % 4) * 128
                jd = junkp.tile([128, 128], F32, tag="jd")
                nc.vector.tensor_tensor(jd[:], pg[:, off:off + 128],
                                        ident[:], ALU.mult)
                d = dtmpp.tile([128, 1], F32, tag="d")
                nc.vector.reduce_sum(d[:], jd[:], axis=mybir.AxisListType.X)
                if scale_exp:
                    nc.scalar.activation(dst_ap, d[:], AF.Exp, scale=EXP_SC)
                else:
                    nc.vector.tensor_scalar_mul(dst_ap, d[:], EXP_SC)

            with tc.tile_pool(name="rhs", bufs=8) as rhsp, \
                 tc.tile_pool(name="junk", bufs=4) as junkp, \
                 tc.tile_pool(name="jk8p", bufs=6) as jk8p, \
                 tc.tile_pool(name="dtmp", bufs=4) as dtmpp, \
                 tc.tile_pool(name="ps2", bufs=1, space="PSUM") as ps2:

                # local-block prelude (d=0, both col halves) — needs no AG
                for k in range(2):
                    for m in range(MT):
                        pg = ps2.tile([128, 512], F32, tag="pg", bufs=6,
                                      name=f"pgl{k}_{m}")
                        for t in range(4):
                            dr_gemm(pg, m, znT8[:, k, 2 * t:2 * t + 2, :], t)
                        junk = junkp.tile([128, 512], BF16, tag="junk")
                        nc.scalar.activation(
                            junk[:], pg[:], AF.Exp, scale=EXP_SC,
                            accum_out=rowsum[:, m, k:k + 1])
                        if k == m // 4:
                            diag_to(corr_acc[:, m:m + 1], pg, m, True,
                                    junkp, dtmpp)

                # prefetch all remote column blocks; 4KB contiguous per
                # partition on both sides so the DMA runs at full rate.
                rhs_t = {}
                for k in range(2):
                    for dd in range(1, 5):
                        row0 = ((pid + dd) % N_CORES) * 128
                        rhs = rhsp.tile([128, KT, 512], FP8, tag="rhs",
                                        name=f"rhs{k}_{dd}")
                        src = ag_out[k][bass.ds(row0, 128), :].rearrange(
                            "p (kt j) -> p kt j", kt=KT)
                        nc.sync.dma_start(rhs[:], src)
                        rhs_t[(k, dd)] = rhs

                def remote_pass(k, dds):
                    jk8 = {}
                    for dd in dds:
                        if dd < 4:
                            jk8[dd] = jk8p.tile([128, MT, 512], FP8E5,
                                                tag="jk8",
                                                name=f"jk8_{k}_{dd}")
                    for m in range(MT):
                        pgs = {dd: ps2.tile([128, 512], F32, tag="pg",
                                            bufs=6, name=f"pg{k}_{dd}_{m}")
                               for dd in dds}
                        for t in range(4):
                            for dd in dds:
                                dr_gemm(pgs[dd], m,
                                        rhs_t[(k, dd)][:, 2 * t:2 * t + 2, :],
                                        t)
                        for dd in dds:
                            slot = 2 + (dd - 1) * 2 + k
                            if dd < 4:
                                nc.scalar.activation(
                                    jk8[dd][:, m, :], pgs[dd][:], AF.Exp,
                                    scale=EXP_SC,
                                    accum_out=rowsum[:, m, slot:slot + 1])
                            else:
                                junk = junkp.tile([128, 512], BF16,
                                                  tag="junk")
                                nc.scalar.activation(
                                    junk[:], pgs[dd][:], AF.Exp,
                                    scale=EXP_SC,
                                    accum_out=rowsum[:, m, slot:slot + 1])
                                if k == m // 4:
                                    diag_to(pos_acc[:, m:m + 1], pgs[dd], m,
                                            False, junkp, dtmpp)
                    # column sums for cores (c+dd): DoubleRow ones-matmuls
                    for dd in dds:
                        if dd >= 4:
                            continue
                        cs = ps2.tile([128, 512], F32, tag="cs", bufs=1,
                                      name=f"cs{k}_{dd}")
                        for q in range(4):
                            nc.tensor.matmul(
                                cs[:], ones_e52[:],
                                jk8[dd][:, 2 * q:2 * q + 2, :],
                                start=(q == 0), stop=(q == 3), perf_mode=DR)
                        css = dtmpp.tile([1, 512], F32, tag="css",
                                         name=f"css{k}_{dd}")
                        nc.vector.tensor_copy(css[:], cs[:1, :])
                        nc.gpsimd.dma_start(
                            cs_in[k][dd - 1:dd, :], css[:])

                def cs_exchange(k):
                    if no_collective:
                        for c in range(N_CORES):
                            nc.sync.dma_start(
                                cs_out[k][c * 3:(c + 1) * 3, :], cs_in[k][:])
                    else:
                        nc.gpsimd.collective_compute(
                            "AllGather", ALU.bypass,
                            replica_groups=[list(range(N_CORES))],
                            ins=[cs_in[k][:]], outs=[cs_out[k][:]])

                remote_pass(0, [1, 2, 3])
                cs_exchange(0)
                remote_pass(0, [4])
                remote_pass(1, [1, 2, 3])
                cs_exchange(1)
                remote_pass(1, [4])

                # ---- finale: lse, nll, partial sum (batched over m) ----
                rs = small.tile([128, MT], F32)
                nc.vector.reduce_sum(rs[:], rowsum[:],
                                     axis=mybir.AxisListType.X)
                nc.vector.tensor_tensor(rs[:], rs[:], corr_acc[:],
                                        ALU.subtract)
                rcv3 = small.tile([128, MT, 3], F32)
                for k in range(2):
                    for dd in range(1, 4):
                        row = ((pid + (N_CORES - dd)) % N_CORES) * 3 \
                            + (dd - 1)
                        nc.sync.dma_start(
                            rcv3[:, 4 * k:4 * (k + 1), dd - 1],
                            cs_out[k][bass.ds(row, 1), :]
                            .rearrange("one (m p) -> (one p) m", p=128))
                rcv = small.tile([128, MT], F32)
                nc.vector.reduce_sum(rcv[:], rcv3[:],
                                     axis=mybir.AxisListType.X)
                nc.vector.tensor_tensor(rs[:], rs[:], rcv[:], ALU.add)
                lse = small.tile([128, MT], F32)
                nc.scalar.activation(lse[:], rs[:], AF.Ln)
                nll = small.tile([128, MT], F32)
                nc.vector.tensor_tensor(nll[:], lse[:], pos_acc[:],
                                        ALU.subtract)
                pf = ps2.tile([1, MT], F32, tag="pf", bufs=1)
                nc.tensor.matmul(pf[:], ones_f[:], nll[:], start=True,
                                 stop=True)
                fs = small.tile([1, MT], F32)
                nc.vector.tensor_copy(fs[:], pf[:])
                nc.scalar.dma_start(out[:], fs[:])

    nc.compile()
    return nc


def kernel(x, w, b):
    if "nc" not in _cached:
        _cached["nc"] = _build()
    nc = _cached["nc"]
    x = np.ascontiguousarray(np.asarray(x, dtype=np.float32))
    w = np.ascontiguousarray(np.asarray(w, dtype=np.float32))
    b = np.ascontiguousarray(np.asarray(b, dtype=np.float32))
    in_maps = [{
        "xs": np.ascontiguousarray(x[c * R:(c + 1) * R]),
        "w": w, "b": b,
    } for c in range(N_CORES)]
    res = run_bass_kernel_spmd(nc, in_maps, list(range(N_CORES)))
    total = 0.0
    for c in range(N_CORES):
        total += float(res.results[c]["out"].astype(np.float64).sum())
    return np.float32(total / B)


# revision 3
# speedup vs baseline: 1.0595x; 1.0595x over previous
"""Contrastive projection head loss on 8 Trainium2 NeuronCores (v10, ~217-226us; v2 baseline was 250-279us).

Reference computation (B=8192, E=1024, P=512):
    z_codon = relu(x[:, :E]) @ w + b          # [B, P]
    z_amino = relu(x[:, E:]) @ w + b          # [B, P]
    z  = concat([z_codon, z_amino], axis=1)   # [B, 2P]
    zn = z / max(||z||, 1e-8)
    s  = (zn @ zn.T);  s[i,i] = -9e15;  s /= 0.1
    nll_i = -s[i, (i - B/2) % B] + logsumexp(s[i, :])
    out = mean(nll)

v10 changes vs v8:
 - zn ship split into two 4-kt chunks per half: the first chunk flies
   while kt4-7 still normalize, pulling each AllGather trigger ~3us
   earlier.
 - cs_out readbacks issued right after each cs AllGather instead of in
   the finale, removing their latency from the tail chain.

v8 changes vs v7:
 - Remote pass order k0(dd1-3), k0(dd4), k1(dd1-3), k1(dd4): all the
   AG0-gated work runs before anything needs AG1, removing the ~8us
   PE stall observed while AG1 was still in flight.

v7 changes vs v6:
 - w staged on the sync ring BETWEEN x row-groups 5 and 6: rows 0-3
   stream uncontended so the jh0 chain (and AG0) launch ~12us earlier;
   w still lands just before the jh0 projections need it.
 - Column-sum exchange split back into two AllGathers, each launched
   right after its k-pass so both hide under the dd=4 passes.

v6 changes vs v5:
 - Dummy 128B AllGather right after the load triggers: the ~14us
   core-launch skew drains during the x stream instead of inflating
   the first real collective.
 - Activation-table warmups removed (tables evict on function switch,
   so they bought nothing and cost ~4us of early scalar time).

v5 changes vs v4 (261us) / v3 (225us):
 - ALL x DMA triggers on the sync queue (one ring sustains the ~260GB/s
   per-core HBM rate; putting half on scalar let ring backpressure
   block scalar compute behind the trigger slices until ~25us).
 - xrow pool bufs=8: no buffer-reuse waits.
 - Back to two per-half 512KB AllGathers (the single 1MB AG cost 43us
   of CC time and stalled the PE 68us; two small ones pipeline).
 - PSUM transpose staging [128, 16, 128] (2 banks): one eviction copy
   per row-group.
 - Engine split tuned to measured rates: relu V/S alternating, big
   copies V/S alternating, sq and normalize Vector/GpSimd alternating.

Returns per-core partial sums [1, 8]; host sums and divides by B.
"""
import numpy as np

from concourse import bass, mybir, tile, bacc
from concourse.bass_utils import run_bass_kernel_spmd
from concourse.masks import make_identity

N_CORES = 8
B = 8192
E = 1024          # embedding size (per half)
P = 512           # projection size
D = 2 * P         # z feature dim = 1024
R = B // N_CORES  # rows per core = 1024
KT = D // 128     # feature sub-tiles = 8
MT = R // 128     # row sub-tiles per core = 8
INV_T = 10.0      # 1 / temperature
SC_Z = 16.0       # zn pre-scale before fp8 cast
SC_W = 32.0       # w pre-scale before fp8 cast
EXP_SC = INV_T / (SC_Z * SC_Z)   # activation scale recovering s/T

F32 = mybir.dt.float32
F32R = mybir.dt.float32r
BF16 = mybir.dt.bfloat16
FP8 = mybir.dt.float8e4
FP8E5 = mybir.dt.float8e5
AF = mybir.ActivationFunctionType
ALU = mybir.AluOpType
DR = mybir.MatmulPerfMode.DoubleRow

NSLOT = 10        # rowsum slots: 2 local (d=0) + 8 remote (d=1..4, k=0..1)

_cached = {}


def _build(no_collective=False):
    nc = bacc.Bacc("TRN2", target_bir_lowering=False, debug=False,
                   enable_asserts=False, num_devices=N_CORES)
    x_in = nc.dram_tensor("xs", [R, 2 * E], F32, kind="ExternalInput").ap()
    w_in = nc.dram_tensor("w", [E, P], F32, kind="ExternalInput").ap()
    b_in = nc.dram_tensor("b", [P], F32, kind="ExternalInput").ap()
    out = nc.dram_tensor("out", [1, MT], F32, kind="ExternalOutput").ap()

    with tile.TileContext(nc) as tc:
        with tc.tile_pool(name="const", bufs=1) as const, \
             tc.tile_pool(name="big", bufs=1) as big, \
             tc.tile_pool(name="small", bufs=1) as small, \
             tc.tile_pool(name="dram", bufs=1, space="DRAM") as dram:

            ident = const.tile([128, 128], F32)
            make_identity(nc, ident[:])
            idb = const.tile([128, 128], BF16)
            make_identity(nc, idb[:])
            ones_f = const.tile([128, 1], F32)
            nc.vector.memset(ones_f[:], 1.0)
            ones_r = const.tile([128, 1], F32R)
            nc.vector.tensor_copy(ones_r[:], ones_f[:])
            ones_f1 = const.tile([1, 128], F32)
            nc.vector.memset(ones_f1[:], 1.0)
            ones_r1 = const.tile([1, 128], F32R)
            nc.vector.tensor_copy(ones_r1[:], ones_f1[:])
            ones_e52 = const.tile([128, 2, 128], FP8E5)
            nc.vector.memset(ones_e52[:], 1.0)
            b2 = const.tile([128, P // 128], F32)
            nc.sync.dma_start(b2[:], b_in.rearrange("(mt p) -> p mt", p=128))

            # w as [128, KT(=E/128), P] fp8, scaled x32
            w8 = const.tile([128, E // 128, P], FP8)

            # znT8: zn x16, fp8, jh-major feature-major [128, 2, KT, 512]
            znT8 = big.tile([128, 2, KT, 512], FP8, tag="zn8")
            ag_in = [dram.tile([128, KT * 512], FP8, name=f"ag_in{k}")
                     for k in range(2)]
            ag_out = [dram.tile([N_CORES * 128, KT * 512], FP8,
                                name=f"ag_out{k}",
                                addr_space="Local" if no_collective else "Shared")
                      for k in range(2)]
            dummy_in = dram.tile([1, 128], FP8, name="dummy_in")
            dummy_out = dram.tile([N_CORES, 128], FP8, name="dummy_out",
                                  addr_space="Local" if no_collective
                                  else "Shared")
            cs_in = [dram.tile([3, 512], F32, name=f"cs_in{k}")
                     for k in range(2)]
            cs_out = [dram.tile([N_CORES * 3, 512], F32, name=f"cs_out{k}",
                                addr_space="Local" if no_collective
                                else "Shared")
                      for k in range(2)]

            # ---- phase 1 ----
            with tc.tile_pool(name="xrow", bufs=8) as xrowp, \
                 tc.tile_pool(name="wst", bufs=1) as wstp, \
                 tc.tile_pool(name="x16", bufs=3) as x16p, \
                 tc.tile_pool(name="xT", bufs=2) as xTp, \
                 tc.tile_pool(name="zT", bufs=2) as zTp, \
                 tc.tile_pool(name="sq", bufs=2) as sqp, \
                 tc.tile_pool(name="rns", bufs=4) as rnsp, \
                 tc.tile_pool(name="ps1", bufs=1, space="PSUM") as ps1:

                # all x loads queued up front; 8 buffers so no trigger
                # ever waits on compute.
                xrows = []
                wstage = wstp.tile([128, E // 128, P], F32, tag="wstage")
                for rg in range(8):
                    xrow = xrowp.tile([128, 2 * E], F32, tag="xrow")
                    nc.sync.dma_start(xrow[:],
                                      x_in[rg * 128:(rg + 1) * 128, :])
                    xrows.append(xrow)
                    if rg == 5:
                        # w lands after rows 0-5, just before the jh0
                        # projections need it
                        nc.sync.dma_start(
                            wstage[:],
                            w_in.rearrange("(kt p) q -> p kt q", p=128))
                pid = nc.sync.partition_id()
                # dummy collective: absorbs core-launch skew while the x
                # stream runs, so the first real AllGather starts promptly
                if not no_collective:
                    nc.gpsimd.collective_compute(
                        "AllGather", ALU.bypass,
                        replica_groups=[list(range(N_CORES))],
                        ins=[dummy_in[:]], outs=[dummy_out[:]])

                for jh in range(2):
                    xT8 = xTp.tile([128, 2 * KT, 512], FP8, tag="xT",
                                   name=f"xT{jh}")
                    for r in range(4):
                        rg = jh * 4 + r
                        x16 = x16p.tile([128, 2 * E], BF16, tag="x16")
                        if rg % 2 == 0:
                            nc.vector.tensor_scalar_max(x16[:], xrows[rg][:],
                                                        0.0)
                        else:
                            nc.scalar.activation(x16[:], xrows[rg][:], AF.Relu)
                        # all 16 feature tiles transpose into one 2-bank
                        # psum tile; a single eviction copy per row-group
                        pt = ps1.tile([128, 16, 128], BF16, tag="pt",
                                      bufs=2)
                        for ct in range(16):
                            nc.tensor.transpose(
                                pt[:, ct, :],
                                x16[:, ct * 128:(ct + 1) * 128],
                                idb[:])
                        dst = xT8[:, :, r * 128:(r + 1) * 128]
                        if rg % 2 == 0:
                            nc.vector.tensor_copy(dst, pt[:])
                        else:
                            nc.scalar.activation(dst, pt[:], AF.Identity)
                    if jh == 0:
                        nc.vector.tensor_scalar_mul(w8[:], wstage[:], SC_W)
                    # project this half with DoubleRow fp8; interleave the
                    # norm accumulation so sqrt can fire right after the
                    # last projection tile.
                    zT = zTp.tile([128, KT, 512], F32, tag="zT",
                                  name=f"zT{jh}")
                    pn = ps1.tile([1, 512], F32, tag="pn", bufs=1,
                                  name=f"pn{jh}")
                    for h in range(2):
                        for m4 in range(P // 128):
                            pz = ps1.tile([128, 512], F32, tag="pz", bufs=2)
                            for t in range(4):
                                nc.tensor.matmul(
                                    pz[:],
                                    w8[:, 2 * t:2 * t + 2,
                                       m4 * 128:(m4 + 1) * 128],
                                    xT8[:, h * KT + 2 * t:h * KT + 2 * t + 2,
                                        :],
                                    start=(t == 0), stop=(t == 3),
                                    perf_mode=DR)
                            kt = h * 4 + m4
                            nc.scalar.activation(
                                zT[:, kt, :], pz[:], AF.Identity,
                                bias=b2[:, m4:m4 + 1], scale=1.0 / SC_W)
                            sq = sqp.tile([128, 512], F32R, tag="sq")
                            eng = nc.vector if kt % 2 == 0 else nc.gpsimd
                            eng.tensor_tensor(sq[:], zT[:, kt, :],
                                              zT[:, kt, :], ALU.mult)
                            nc.tensor.matmul(pn[:], ones_r[:], sq[:],
                                             start=(kt == 0), stop=(kt == 7))
                    nrm = rnsp.tile([1, 512], F32, tag="nrm", name=f"nrm{jh}")
                    nc.scalar.activation(nrm[:], pn[:], AF.Sqrt,
                                         scale=1.0 / (SC_Z * SC_Z))
                    rn = rnsp.tile([1, 512], F32, tag="rn", name=f"rn{jh}")
                    nc.vector.reciprocal_approx_fast(rn[:], nrm[:])
                    rnr = rnsp.tile([1, 512], F32R, tag="rnr", name=f"rnr{jh}")
                    nc.vector.tensor_copy(rnr[:], rn[:])
                    # broadcast rn across partitions on the PE (K=1 matmul)
                    rnp = ps1.tile([128, 512], F32, tag="rnp", bufs=1,
                                   name=f"rnp{jh}")
                    nc.tensor.matmul(rnp[:], ones_r1[:], rnr[:],
                                     start=True, stop=True)
                    # GpSimd can't read PSUM; land the broadcast in SBUF
                    rnb = rnsp.tile([128, 512], F32, tag="rnb",
                                    name=f"rnb{jh}")
                    nc.vector.tensor_copy(rnb[:], rnp[:])
                    for kt in range(KT):
                        eng = nc.vector if kt % 2 == 0 else nc.gpsimd
                        eng.tensor_tensor(znT8[:, jh, kt, :], zT[:, kt, :],
                                          rnb[:], ALU.mult)
                    # ship this half in two chunks: the first flies while
                    # kt4-7 still normalize
                    agv = ag_in[jh].rearrange("p (kt j) -> p kt j", kt=KT)
                    nc.scalar.dma_start(agv[:, :4], znT8[:, jh, :4])
                    nc.scalar.dma_start(agv[:, 4:], znT8[:, jh, 4:])
                    if no_collective:
                        for c in range(N_CORES):
                            nc.sync.dma_start(
                                ag_out[jh][c * 128:(c + 1) * 128, :],
                                ag_in[jh][:])
                    else:
                        nc.gpsimd.collective_compute(
                            "AllGather", ALU.bypass,
                            replica_groups=[list(range(N_CORES))],
                            ins=[ag_in[jh][:]], outs=[ag_out[jh][:]])

            # ---- phase 2: symmetric blockwise cos-sim ----
            rowsum = const.tile([128, MT, NSLOT], F32)
            pos_acc = const.tile([128, MT], F32)
            corr_acc = const.tile([128, MT], F32)

            def stat(m, t):
                return znT8[:, m // 4, 2 * t:2 * t + 2,
                            (m % 4) * 128:(m % 4 + 1) * 128]

            def dr_gemm(pg, m, rhs_ap, t):
                nc.tensor.matmul(pg[:], stat(m, t), rhs_ap,
                                 start=(t == 0), stop=(t == 3), perf_mode=DR)

            def diag_to(dst_ap, pg, m, scale_exp, junkp, dtmpp):
                off = (m % 4) * 128
                jd = junkp.tile([128, 128], F32, tag="jd")
                nc.vector.tensor_tensor(jd[:], pg[:, off:off + 128],
                                        ident[:], ALU.mult)
                d = dtmpp.tile([128, 1], F32, tag="d")
                nc.vector.reduce_sum(d[:], jd[:], axis=mybir.AxisListType.X)
                if scale_exp:
                    nc.scalar.activation(dst_ap, d[:], AF.Exp, scale=EXP_SC)
                else:
                    nc.vector.tensor_scalar_mul(dst_ap, d[:], EXP_SC)

            with tc.tile_pool(name="rhs", bufs=8) as rhsp, \
                 tc.tile_pool(name="junk", bufs=4) as junkp, \
                 tc.tile_pool(name="jk8p", bufs=6) as jk8p, \
                 tc.tile_pool(name="dtmp", bufs=4) as dtmpp, \
                 tc.tile_pool(name="ps2", bufs=1, space="PSUM") as ps2:

                # local-block prelude (d=0, both col halves) — needs no AG
                for k in range(2):
                    for m in range(MT):
                        pg = ps2.tile([128, 512], F32, tag="pg", bufs=6,
                                      name=f"pgl{k}_{m}")
                        for t in range(4):
                            dr_gemm(pg, m, znT8[:, k, 2 * t:2 * t + 2, :], t)
                        junk = junkp.tile([128, 512], BF16, tag="junk")
                        nc.scalar.activation(
                            junk[:], pg[:], AF.Exp, scale=EXP_SC,
                            accum_out=rowsum[:, m, k:k + 1])
                        if k == m // 4:
                            diag_to(corr_acc[:, m:m + 1], pg, m, True,
                                    junkp, dtmpp)

                # prefetch all remote column blocks; 4KB contiguous per
                # partition on both sides so the DMA runs at full rate.
                rhs_t = {}
                for k in range(2):
                    for dd in range(1, 5):
                        row0 = ((pid + dd) % N_CORES) * 128
                        rhs = rhsp.tile([128, KT, 512], FP8, tag="rhs",
                                        name=f"rhs{k}_{dd}")
                        src = ag_out[k][bass.ds(row0, 128), :].rearrange(
                            "p (kt j) -> p kt j", kt=KT)
                        nc.sync.dma_start(rhs[:], src)
                        rhs_t[(k, dd)] = rhs

                def remote_pass(k, dds):
                    jk8 = {}
                    for dd in dds:
                        if dd < 4:
                            jk8[dd] = jk8p.tile([128, MT, 512], FP8E5,
                                                tag="jk8",
                                                name=f"jk8_{k}_{dd}")
                    for m in range(MT):
                        pgs = {dd: ps2.tile([128, 512], F32, tag="pg",
                                            bufs=6, name=f"pg{k}_{dd}_{m}")
                               for dd in dds}
                        for t in range(4):
                            for dd in dds:
                                dr_gemm(pgs[dd], m,
                                        rhs_t[(k, dd)][:, 2 * t:2 * t + 2, :],
                                        t)
                        for dd in dds:
                            slot = 2 + (dd - 1) * 2 + k
                            if dd < 4:
                                nc.scalar.activation(
                                    jk8[dd][:, m, :], pgs[dd][:], AF.Exp,
                                    scale=EXP_SC,
                                    accum_out=rowsum[:, m, slot:slot + 1])
                            else:
                                junk = junkp.tile([128, 512], BF16,
                                                  tag="junk")
                                nc.scalar.activation(
                                    junk[:], pgs[dd][:], AF.Exp,
                                    scale=EXP_SC,
                                    accum_out=rowsum[:, m, slot:slot + 1])
                                if k == m // 4:
                                    diag_to(pos_acc[:, m:m + 1], pgs[dd], m,
                                            False, junkp, dtmpp)
                    # column sums for cores (c+dd): DoubleRow ones-matmuls
                    for dd in dds:
                        if dd >= 4:
                            continue
                        cs = ps2.tile([128, 512], F32, tag="cs", bufs=1,
                                      name=f"cs{k}_{dd}")
                        for q in range(4):
                            nc.tensor.matmul(
                                cs[:], ones_e52[:],
                                jk8[dd][:, 2 * q:2 * q + 2, :],
                                start=(q == 0), stop=(q == 3), perf_mode=DR)
                        css = dtmpp.tile([1, 512], F32, tag="css",
                                         name=f"css{k}_{dd}")
                        nc.vector.tensor_copy(css[:], cs[:1, :])
                        nc.gpsimd.dma_start(
                            cs_in[k][dd - 1:dd, :], css[:])

                def cs_exchange(k):
                    if no_collective:
                        for c in range(N_CORES):
                            nc.sync.dma_start(
                                cs_out[k][c * 3:(c + 1) * 3, :], cs_in[k][:])
                    else:
                        nc.gpsimd.collective_compute(
                            "AllGather", ALU.bypass,
                            replica_groups=[list(range(N_CORES))],
                            ins=[cs_in[k][:]], outs=[cs_out[k][:]])

                rcv3 = small.tile([128, MT, 3], F32)

                def rcv_load(k):
                    for dd in range(1, 4):
                        row = ((pid + (N_CORES - dd)) % N_CORES) * 3 \
                            + (dd - 1)
                        nc.sync.dma_start(
                            rcv3[:, 4 * k:4 * (k + 1), dd - 1],
                            cs_out[k][bass.ds(row, 1), :]
                            .rearrange("one (m p) -> (one p) m", p=128))

                remote_pass(0, [1, 2, 3])
                cs_exchange(0)
                rcv_load(0)
                remote_pass(0, [4])
                remote_pass(1, [1, 2, 3])
                cs_exchange(1)
                rcv_load(1)
                remote_pass(1, [4])

                # ---- finale: lse, nll, partial sum (batched over m) ----
                rs = small.tile([128, MT], F32)
                nc.vector.reduce_sum(rs[:], rowsum[:],
                                     axis=mybir.AxisListType.X)
                nc.vector.tensor_tensor(rs[:], rs[:], corr_acc[:],
                                        ALU.subtract)
                rcv = small.tile([128, MT], F32)
                nc.vector.reduce_sum(rcv[:], rcv3[:],
                                     axis=mybir.AxisListType.X)
                nc.vector.tensor_tensor(rs[:], rs[:], rcv[:], ALU.add)
                lse = small.tile([128, MT], F32)
                nc.scalar.activation(lse[:], rs[:], AF.Ln)
                nll = small.tile([128, MT], F32)
                nc.vector.tensor_tensor(nll[:], lse[:], pos_acc[:],
                                        ALU.subtract)
                pf = ps2.tile([1, MT], F32, tag="pf", bufs=1)
                nc.tensor.matmul(pf[:], ones_f[:], nll[:], start=True,
                                 stop=True)
                fs = small.tile([1, MT], F32)
                nc.vector.tensor_copy(fs[:], pf[:])
                nc.scalar.dma_start(out[:], fs[:])

    nc.compile()
    return nc


def kernel(x, w, b):
    if "nc" not in _cached:
        _cached["nc"] = _build()
    nc = _cached["nc"]
    x = np.ascontiguousarray(np.asarray(x, dtype=np.float32))
    w = np.ascontiguousarray(np.asarray(w, dtype=np.float32))
    b = np.ascontiguousarray(np.asarray(b, dtype=np.float32))
    in_maps = [{
        "xs": np.ascontiguousarray(x[c * R:(c + 1) * R]),
        "w": w, "b": b,
    } for c in range(N_CORES)]
    res = run_bass_kernel_spmd(nc, in_maps, list(range(N_CORES)))
    total = 0.0
    for c in range(N_CORES):
        total += float(res.results[c]["out"].astype(np.float64).sum())
    return np.float32(total / B)


# revision 4
# speedup vs baseline: 1.1263x; 1.0631x over previous
"""Contrastive projection head loss on 8 Trainium2 NeuronCores (v11, ~196-219us; v2 baseline was 250-279us).

Reference computation (B=8192, E=1024, P=512):
    z_codon = relu(x[:, :E]) @ w + b          # [B, P]
    z_amino = relu(x[:, E:]) @ w + b          # [B, P]
    z  = concat([z_codon, z_amino], axis=1)   # [B, 2P]
    zn = z / max(||z||, 1e-8)
    s  = (zn @ zn.T);  s[i,i] = -9e15;  s /= 0.1
    nll_i = -s[i, (i - B/2) % B] + logsumexp(s[i, :])
    out = mean(nll)

v11 changes vs v10:
 - Vector normalizes read the rn broadcast straight from PSUM; the
   SBUF copy (for GpSimd, which can't read PSUM) moved to Scalar so
   both engine streams start ~1us earlier per half.
 - Finale reduction over rowsum slots 0-8 precomputed before the last
   remote pass completes; only slot 9 and the log remain on the tail.

v10 changes vs v8:
 - zn ship split into two 4-kt chunks per half: the first chunk flies
   while kt4-7 still normalize, pulling each AllGather trigger ~3us
   earlier.
 - cs_out readbacks issued right after each cs AllGather instead of in
   the finale, removing their latency from the tail chain.

v8 changes vs v7:
 - Remote pass order k0(dd1-3), k0(dd4), k1(dd1-3), k1(dd4): all the
   AG0-gated work runs before anything needs AG1, removing the ~8us
   PE stall observed while AG1 was still in flight.

v7 changes vs v6:
 - w staged on the sync ring BETWEEN x row-groups 5 and 6: rows 0-3
   stream uncontended so the jh0 chain (and AG0) launch ~12us earlier;
   w still lands just before the jh0 projections need it.
 - Column-sum exchange split back into two AllGathers, each launched
   right after its k-pass so both hide under the dd=4 passes.

v6 changes vs v5:
 - Dummy 128B AllGather right after the load triggers: the ~14us
   core-launch skew drains during the x stream instead of inflating
   the first real collective.
 - Activation-table warmups removed (tables evict on function switch,
   so they bought nothing and cost ~4us of early scalar time).

v5 changes vs v4 (261us) / v3 (225us):
 - ALL x DMA triggers on the sync queue (one ring sustains the ~260GB/s
   per-core HBM rate; putting half on scalar let ring backpressure
   block scalar compute behind the trigger slices until ~25us).
 - xrow pool bufs=8: no buffer-reuse waits.
 - Back to two per-half 512KB AllGathers (the single 1MB AG cost 43us
   of CC time and stalled the PE 68us; two small ones pipeline).
 - PSUM transpose staging [128, 16, 128] (2 banks): one eviction copy
   per row-group.
 - Engine split tuned to measured rates: relu V/S alternating, big
   copies V/S alternating, sq and normalize Vector/GpSimd alternating.

Returns per-core partial sums [1, 8]; host sums and divides by B.
"""
import numpy as np

from concourse import bass, mybir, tile, bacc
from concourse.bass_utils import run_bass_kernel_spmd
from concourse.masks import make_identity

N_CORES = 8
B = 8192
E = 1024          # embedding size (per half)
P = 512           # projection size
D = 2 * P         # z feature dim = 1024
R = B // N_CORES  # rows per core = 1024
KT = D // 128     # feature sub-tiles = 8
MT = R // 128     # row sub-tiles per core = 8
INV_T = 10.0      # 1 / temperature
SC_Z = 16.0       # zn pre-scale before fp8 cast
SC_W = 32.0       # w pre-scale before fp8 cast
EXP_SC = INV_T / (SC_Z * SC_Z)   # activation scale recovering s/T

F32 = mybir.dt.float32
F32R = mybir.dt.float32r
BF16 = mybir.dt.bfloat16
FP8 = mybir.dt.float8e4
FP8E5 = mybir.dt.float8e5
AF = mybir.ActivationFunctionType
ALU = mybir.AluOpType
DR = mybir.MatmulPerfMode.DoubleRow

NSLOT = 10        # rowsum slots: 2 local (d=0) + 8 remote (d=1..4, k=0..1)

_cached = {}


def _build(no_collective=False):
    nc = bacc.Bacc("TRN2", target_bir_lowering=False, debug=False,
                   enable_asserts=False, num_devices=N_CORES)
    x_in = nc.dram_tensor("xs", [R, 2 * E], F32, kind="ExternalInput").ap()
    w_in = nc.dram_tensor("w", [E, P], F32, kind="ExternalInput").ap()
    b_in = nc.dram_tensor("b", [P], F32, kind="ExternalInput").ap()
    out = nc.dram_tensor("out", [1, MT], F32, kind="ExternalOutput").ap()

    with tile.TileContext(nc) as tc:
        with tc.tile_pool(name="const", bufs=1) as const, \
             tc.tile_pool(name="big", bufs=1) as big, \
             tc.tile_pool(name="small", bufs=1) as small, \
             tc.tile_pool(name="dram", bufs=1, space="DRAM") as dram:

            ident = const.tile([128, 128], F32)
            make_identity(nc, ident[:])
            idb = const.tile([128, 128], BF16)
            make_identity(nc, idb[:])
            ones_f = const.tile([128, 1], F32)
            nc.vector.memset(ones_f[:], 1.0)
            ones_r = const.tile([128, 1], F32R)
            nc.vector.tensor_copy(ones_r[:], ones_f[:])
            ones_f1 = const.tile([1, 128], F32)
            nc.vector.memset(ones_f1[:], 1.0)
            ones_r1 = const.tile([1, 128], F32R)
            nc.vector.tensor_copy(ones_r1[:], ones_f1[:])
            ones_e52 = const.tile([128, 2, 128], FP8E5)
            nc.vector.memset(ones_e52[:], 1.0)
            b2 = const.tile([128, P // 128], F32)
            nc.sync.dma_start(b2[:], b_in.rearrange("(mt p) -> p mt", p=128))

            # w as [128, KT(=E/128), P] fp8, scaled x32
            w8 = const.tile([128, E // 128, P], FP8)

            # znT8: zn x16, fp8, jh-major feature-major [128, 2, KT, 512]
            znT8 = big.tile([128, 2, KT, 512], FP8, tag="zn8")
            ag_in = [dram.tile([128, KT * 512], FP8, name=f"ag_in{k}")
                     for k in range(2)]
            ag_out = [dram.tile([N_CORES * 128, KT * 512], FP8,
                                name=f"ag_out{k}",
                                addr_space="Local" if no_collective else "Shared")
                      for k in range(2)]
            dummy_in = dram.tile([1, 128], FP8, name="dummy_in")
            dummy_out = dram.tile([N_CORES, 128], FP8, name="dummy_out",
                                  addr_space="Local" if no_collective
                                  else "Shared")
            cs_in = [dram.tile([3, 512], F32, name=f"cs_in{k}")
                     for k in range(2)]
            cs_out = [dram.tile([N_CORES * 3, 512], F32, name=f"cs_out{k}",
                                addr_space="Local" if no_collective
                                else "Shared")
                      for k in range(2)]

            # ---- phase 1 ----
            with tc.tile_pool(name="xrow", bufs=8) as xrowp, \
                 tc.tile_pool(name="wst", bufs=1) as wstp, \
                 tc.tile_pool(name="x16", bufs=3) as x16p, \
                 tc.tile_pool(name="xT", bufs=2) as xTp, \
                 tc.tile_pool(name="zT", bufs=2) as zTp, \
                 tc.tile_pool(name="sq", bufs=2) as sqp, \
                 tc.tile_pool(name="rns", bufs=4) as rnsp, \
                 tc.tile_pool(name="ps1", bufs=1, space="PSUM") as ps1:

                # all x loads queued up front; 8 buffers so no trigger
                # ever waits on compute.
                xrows = []
                wstage = wstp.tile([128, E // 128, P], F32, tag="wstage")
                for rg in range(8):
                    xrow = xrowp.tile([128, 2 * E], F32, tag="xrow")
                    nc.sync.dma_start(xrow[:],
                                      x_in[rg * 128:(rg + 1) * 128, :])
                    xrows.append(xrow)
                    if rg == 5:
                        # w lands after rows 0-5, just before the jh0
                        # projections need it
                        nc.sync.dma_start(
                            wstage[:],
                            w_in.rearrange("(kt p) q -> p kt q", p=128))
                pid = nc.sync.partition_id()
                # dummy collective: absorbs core-launch skew while the x
                # stream runs, so the first real AllGather starts promptly
                if not no_collective:
                    nc.gpsimd.collective_compute(
                        "AllGather", ALU.bypass,
                        replica_groups=[list(range(N_CORES))],
                        ins=[dummy_in[:]], outs=[dummy_out[:]])

                for jh in range(2):
                    xT8 = xTp.tile([128, 2 * KT, 512], FP8, tag="xT",
                                   name=f"xT{jh}")
                    for r in range(4):
                        rg = jh * 4 + r
                        x16 = x16p.tile([128, 2 * E], BF16, tag="x16")
                        if rg % 2 == 0:
                            nc.vector.tensor_scalar_max(x16[:], xrows[rg][:],
                                                        0.0)
                        else:
                            nc.scalar.activation(x16[:], xrows[rg][:], AF.Relu)
                        # all 16 feature tiles transpose into one 2-bank
                        # psum tile; a single eviction copy per row-group
                        pt = ps1.tile([128, 16, 128], BF16, tag="pt",
                                      bufs=2)
                        for ct in range(16):
                            nc.tensor.transpose(
                                pt[:, ct, :],
                                x16[:, ct * 128:(ct + 1) * 128],
                                idb[:])
                        dst = xT8[:, :, r * 128:(r + 1) * 128]
                        if rg % 2 == 0:
                            nc.vector.tensor_copy(dst, pt[:])
                        else:
                            nc.scalar.activation(dst, pt[:], AF.Identity)
                    if jh == 0:
                        nc.vector.tensor_scalar_mul(w8[:], wstage[:], SC_W)
                    # project this half with DoubleRow fp8; interleave the
                    # norm accumulation so sqrt can fire right after the
                    # last projection tile.
                    zT = zTp.tile([128, KT, 512], F32, tag="zT",
                                  name=f"zT{jh}")
                    pn = ps1.tile([1, 512], F32, tag="pn", bufs=1,
                                  name=f"pn{jh}")
                    for h in range(2):
                        for m4 in range(P // 128):
                            pz = ps1.tile([128, 512], F32, tag="pz", bufs=2)
                            for t in range(4):
                                nc.tensor.matmul(
                                    pz[:],
                                    w8[:, 2 * t:2 * t + 2,
                                       m4 * 128:(m4 + 1) * 128],
                                    xT8[:, h * KT + 2 * t:h * KT + 2 * t + 2,
                                        :],
                                    start=(t == 0), stop=(t == 3),
                                    perf_mode=DR)
                            kt = h * 4 + m4
                            nc.scalar.activation(
                                zT[:, kt, :], pz[:], AF.Identity,
                                bias=b2[:, m4:m4 + 1], scale=1.0 / SC_W)
                            sq = sqp.tile([128, 512], F32R, tag="sq")
                            eng = nc.vector if kt % 2 == 0 else nc.gpsimd
                            eng.tensor_tensor(sq[:], zT[:, kt, :],
                                              zT[:, kt, :], ALU.mult)
                            nc.tensor.matmul(pn[:], ones_r[:], sq[:],
                                             start=(kt == 0), stop=(kt == 7))
                    nrm = rnsp.tile([1, 512], F32, tag="nrm", name=f"nrm{jh}")
                    nc.scalar.activation(nrm[:], pn[:], AF.Sqrt,
                                         scale=1.0 / (SC_Z * SC_Z))
                    rn = rnsp.tile([1, 512], F32, tag="rn", name=f"rn{jh}")
                    nc.vector.reciprocal_approx_fast(rn[:], nrm[:])
                    rnr = rnsp.tile([1, 512], F32R, tag="rnr", name=f"rnr{jh}")
                    nc.vector.tensor_copy(rnr[:], rn[:])
                    # broadcast rn across partitions on the PE (K=1 matmul)
                    rnp = ps1.tile([128, 512], F32, tag="rnp", bufs=1,
                                   name=f"rnp{jh}")
                    nc.tensor.matmul(rnp[:], ones_r1[:], rnr[:],
                                     start=True, stop=True)
                    # GpSimd can't read PSUM: Scalar lands the SBUF copy
                    # for it while Vector reads the PSUM broadcast directly
                    rnb = rnsp.tile([128, 512], F32, tag="rnb",
                                    name=f"rnb{jh}")
                    nc.scalar.activation(rnb[:], rnp[:], AF.Identity)
                    for kt in range(KT):
                        if kt % 2 == 0:
                            nc.vector.tensor_tensor(znT8[:, jh, kt, :],
                                                    zT[:, kt, :], rnp[:],
                                                    ALU.mult)
                        else:
                            nc.gpsimd.tensor_tensor(znT8[:, jh, kt, :],
                                                    zT[:, kt, :], rnb[:],
                                                    ALU.mult)
                    # ship this half in two chunks: the first flies while
                    # kt4-7 still normalize
                    agv = ag_in[jh].rearrange("p (kt j) -> p kt j", kt=KT)
                    nc.scalar.dma_start(agv[:, :4], znT8[:, jh, :4])
                    nc.scalar.dma_start(agv[:, 4:], znT8[:, jh, 4:])
                    if no_collective:
                        for c in range(N_CORES):
                            nc.sync.dma_start(
                                ag_out[jh][c * 128:(c + 1) * 128, :],
                                ag_in[jh][:])
                    else:
                        nc.gpsimd.collective_compute(
                            "AllGather", ALU.bypass,
                            replica_groups=[list(range(N_CORES))],
                            ins=[ag_in[jh][:]], outs=[ag_out[jh][:]])

            # ---- phase 2: symmetric blockwise cos-sim ----
            rowsum = const.tile([128, MT, NSLOT], F32)
            pos_acc = const.tile([128, MT], F32)
            corr_acc = const.tile([128, MT], F32)

            def stat(m, t):
                return znT8[:, m // 4, 2 * t:2 * t + 2,
                            (m % 4) * 128:(m % 4 + 1) * 128]

            def dr_gemm(pg, m, rhs_ap, t):
                nc.tensor.matmul(pg[:], stat(m, t), rhs_ap,
                                 start=(t == 0), stop=(t == 3), perf_mode=DR)

            def diag_to(dst_ap, pg, m, scale_exp, junkp, dtmpp):
                off = (m % 4) * 128
                jd = junkp.tile([128, 128], F32, tag="jd")
                nc.vector.tensor_tensor(jd[:], pg[:, off:off + 128],
                                        ident[:], ALU.mult)
                d = dtmpp.tile([128, 1], F32, tag="d")
                nc.vector.reduce_sum(d[:], jd[:], axis=mybir.AxisListType.X)
                if scale_exp:
                    nc.scalar.activation(dst_ap, d[:], AF.Exp, scale=EXP_SC)
                else:
                    nc.vector.tensor_scalar_mul(dst_ap, d[:], EXP_SC)

            with tc.tile_pool(name="rhs", bufs=8) as rhsp, \
                 tc.tile_pool(name="junk", bufs=4) as junkp, \
                 tc.tile_pool(name="jk8p", bufs=6) as jk8p, \
                 tc.tile_pool(name="dtmp", bufs=4) as dtmpp, \
                 tc.tile_pool(name="ps2", bufs=1, space="PSUM") as ps2:

                # local-block prelude (d=0, both col halves) — needs no AG
                for k in range(2):
                    for m in range(MT):
                        pg = ps2.tile([128, 512], F32, tag="pg", bufs=6,
                                      name=f"pgl{k}_{m}")
                        for t in range(4):
                            dr_gemm(pg, m, znT8[:, k, 2 * t:2 * t + 2, :], t)
                        junk = junkp.tile([128, 512], BF16, tag="junk")
                        nc.scalar.activation(
                            junk[:], pg[:], AF.Exp, scale=EXP_SC,
                            accum_out=rowsum[:, m, k:k + 1])
                        if k == m // 4:
                            diag_to(corr_acc[:, m:m + 1], pg, m, True,
                                    junkp, dtmpp)

                # prefetch all remote column blocks; 4KB contiguous per
                # partition on both sides so the DMA runs at full rate.
                rhs_t = {}
                for k in range(2):
                    for dd in range(1, 5):
                        row0 = ((pid + dd) % N_CORES) * 128
                        rhs = rhsp.tile([128, KT, 512], FP8, tag="rhs",
                                        name=f"rhs{k}_{dd}")
                        src = ag_out[k][bass.ds(row0, 128), :].rearrange(
                            "p (kt j) -> p kt j", kt=KT)
                        nc.sync.dma_start(rhs[:], src)
                        rhs_t[(k, dd)] = rhs

                def remote_pass(k, dds):
                    jk8 = {}
                    for dd in dds:
                        if dd < 4:
                            jk8[dd] = jk8p.tile([128, MT, 512], FP8E5,
                                                tag="jk8",
                                                name=f"jk8_{k}_{dd}")
                    for m in range(MT):
                        pgs = {dd: ps2.tile([128, 512], F32, tag="pg",
                                            bufs=6, name=f"pg{k}_{dd}_{m}")
                               for dd in dds}
                        for t in range(4):
                            for dd in dds:
                                dr_gemm(pgs[dd], m,
                                        rhs_t[(k, dd)][:, 2 * t:2 * t + 2, :],
                                        t)
                        for dd in dds:
                            slot = 2 + (dd - 1) * 2 + k
                            if dd < 4:
                                nc.scalar.activation(
                                    jk8[dd][:, m, :], pgs[dd][:], AF.Exp,
                                    scale=EXP_SC,
                                    accum_out=rowsum[:, m, slot:slot + 1])
                            else:
                                junk = junkp.tile([128, 512], BF16,
                                                  tag="junk")
                                nc.scalar.activation(
                                    junk[:], pgs[dd][:], AF.Exp,
                                    scale=EXP_SC,
                                    accum_out=rowsum[:, m, slot:slot + 1])
                                if k == m // 4:
                                    diag_to(pos_acc[:, m:m + 1], pgs[dd], m,
                                            False, junkp, dtmpp)
                    # column sums for cores (c+dd): DoubleRow ones-matmuls
                    for dd in dds:
                        if dd >= 4:
                            continue
                        cs = ps2.tile([128, 512], F32, tag="cs", bufs=1,
                                      name=f"cs{k}_{dd}")
                        for q in range(4):
                            nc.tensor.matmul(
                                cs[:], ones_e52[:],
                                jk8[dd][:, 2 * q:2 * q + 2, :],
                                start=(q == 0), stop=(q == 3), perf_mode=DR)
                        css = dtmpp.tile([1, 512], F32, tag="css",
                                         name=f"css{k}_{dd}")
                        nc.vector.tensor_copy(css[:], cs[:1, :])
                        nc.gpsimd.dma_start(
                            cs_in[k][dd - 1:dd, :], css[:])

                def cs_exchange(k):
                    if no_collective:
                        for c in range(N_CORES):
                            nc.sync.dma_start(
                                cs_out[k][c * 3:(c + 1) * 3, :], cs_in[k][:])
                    else:
                        nc.gpsimd.collective_compute(
                            "AllGather", ALU.bypass,
                            replica_groups=[list(range(N_CORES))],
                            ins=[cs_in[k][:]], outs=[cs_out[k][:]])

                rcv3 = small.tile([128, MT, 3], F32)

                def rcv_load(k):
                    for dd in range(1, 4):
                        row = ((pid + (N_CORES - dd)) % N_CORES) * 3 \
                            + (dd - 1)
                        nc.sync.dma_start(
                            rcv3[:, 4 * k:4 * (k + 1), dd - 1],
                            cs_out[k][bass.ds(row, 1), :]
                            .rearrange("one (m p) -> (one p) m", p=128))

                remote_pass(0, [1, 2, 3])
                cs_exchange(0)
                rcv_load(0)
                remote_pass(0, [4])
                remote_pass(1, [1, 2, 3])
                cs_exchange(1)
                rcv_load(1)
                # everything not gated on the last pass happens now:
                # rowsum slots 0-8, the self-sim correction, and the
                # received column sums
                rs = small.tile([128, MT], F32)
                nc.vector.reduce_sum(rs[:], rowsum[:, :, :NSLOT - 1],
                                     axis=mybir.AxisListType.X)
                nc.vector.tensor_tensor(rs[:], rs[:], corr_acc[:],
                                        ALU.subtract)
                rcv = small.tile([128, MT], F32)
                nc.vector.reduce_sum(rcv[:], rcv3[:],
                                     axis=mybir.AxisListType.X)
                nc.vector.tensor_tensor(rs[:], rs[:], rcv[:], ALU.add)

                remote_pass(1, [4])

                # ---- finale: lse, nll, partial sum (batched over m) ----
                nc.vector.tensor_tensor(rs[:], rs[:],
                                        rowsum[:, :, NSLOT - 1],
                                        ALU.add)
                lse = small.tile([128, MT], F32)
                nc.scalar.activation(lse[:], rs[:], AF.Ln)
                nll = small.tile([128, MT], F32)
                nc.vector.tensor_tensor(nll[:], lse[:], pos_acc[:],
                                        ALU.subtract)
                pf = ps2.tile([1, MT], F32, tag="pf", bufs=1)
                nc.tensor.matmul(pf[:], ones_f[:], nll[:], start=True,
                                 stop=True)
                fs = small.tile([1, MT], F32)
                nc.vector.tensor_copy(fs[:], pf[:])
                nc.scalar.dma_start(out[:], fs[:])

    nc.compile()
    return nc


def kernel(x, w, b):
    if "nc" not in _cached:
        _cached["nc"] = _build()
    nc = _cached["nc"]
    x = np.ascontiguousarray(np.asarray(x, dtype=np.float32))
    w = np.ascontiguousarray(np.asarray(w, dtype=np.float32))
    b = np.ascontiguousarray(np.asarray(b, dtype=np.float32))
    in_maps = [{
        "xs": np.ascontiguousarray(x[c * R:(c + 1) * R]),
        "w": w, "b": b,
    } for c in range(N_CORES)]
    res = run_bass_kernel_spmd(nc, in_maps, list(range(N_CORES)))
    total = 0.0
    for c in range(N_CORES):
        total += float(res.results[c]["out"].astype(np.float64).sum())
    return np.float32(total / B)
